# revision 36
# baseline (speedup 1.0000x reference)
"""Correlation cost-volume kernel for Trainium2 (Bass/Tile), v3.

Problem: in1, in2: [B=8, C=128, H=96, W=128] fp32.
Output: [B, 81, H, W] where out[b, dy*9+dx, y, x] =
    mean_c( in1[b,c,y,x] * in2_pad[b,c,y+dy,x+dx] ),
with in2 zero-padded by 4 in both spatial dims (max_displacement=4).

Data-parallel over batch (one sample per NeuronCore), fp16 compute.

Pipeline per 32-row chunk (4 row-blocks x 8 col-groups of 8x16 pixels,
m = 16r+u):
  1. matmul per block: stationary in1[:, 8rows x 16px] (M=128), moving
     in2p[:, 16 dy', 24 v] (N=384) -> psum[m, dy', v].
  2. ACT/DVE drain psum -> W[m | dy'16, v24, blkg32] fp16 (blkg = 8blk+g
     innermost so the extraction DMAs below get 576B contiguous runs).
  3. u-extract (x-shift): 16 DMAs on Sync (one per u, partitions u::16):
     t2x[m | dy', dx, blkg] = W[m | dy', u+dx, blkg]; src/dst runs 576B+.
  4. r-unshift (y-shift): 8 DMAs on GpSimd (one per r, partitions
     16r..16r+16): t2f[m | dy, dx, blkg] = t2x[m | r+dy, dx, blkg]; one
     contiguous 5.2KB run per partition.
  5. DVE reorder t2f -> t3[m | blkg, 81] so the PE transpose stationary
     has a single contiguous free dim.
  6. PE transpose per block-group: tt[k=81, m] <- t3[:, bg, :].
  7. ACT/DVE drain with 1/C scale to fp16 staging, one contiguous store
     per chunk; host upcasts to fp32.

v3 vs v2: u-first extraction (1.18+0.66 MB/chunk vs 1.77+0.66), Scalar
does no DMAs (was starving the psum drains), all stationary loads are
prefetched upfront, in2p load slices match per-chunk consumption so
chunk-0 matmuls start ~6us in, PSUM pools 3x2-bank mm tiles + 2x1-bank
transpose tiles.
"""

import numpy as np

import concourse.bass as bass
import concourse.mybir as mybir
from concourse import bacc
from concourse.bass_utils import run_bass_kernel_spmd
from concourse.masks import make_identity
from concourse.tile import TileContext

B = 8
C = 128
H = 96
W = 128
D = 9  # 2*max_disp + 1
K = D * D  # 81 output channels
PAD = 4
WP = W + 2 * PAD  # 136
FP32 = mybir.dt.float32
FP16 = mybir.dt.float16

N_CORES = 8
RCH = 48  # rows per chunk
BR = 8  # block rows
BU = 16  # block cols
NBLK = RCH // BR  # 4 row-blocks per chunk
NG = W // BU  # 8 col-groups
NBG = NBLK * NG  # 32 pixel-blocks per chunk
DYP = BR + 2 * PAD  # 16 dy' values per block
VP = BU + 2 * PAD  # 24 v values per group
NMM = DYP * VP  # 384 matmul free size
COPY = mybir.ActivationFunctionType.Copy


def build_bass(h: int = H):
    """Build the per-core Bass program for a [C, h, W] sample."""
    hp = h + 2 * PAD
    nch = h // RCH
    assert h % RCH == 0
    nc = bacc.Bacc(None, target_bir_lowering=False)
    # in1s is host-shuffled to [C, blk_total, g, m] with m = 16r+u,
    # y = 8*blk + r, x = 16*g + u (stationary needs one free dim).
    in1s = nc.dram_tensor("in1s", [C, h // BR, NG, 128], FP16, kind="ExternalInput")
    # in2p is host-padded: [C, h+8, W+8] with zeros in the 4-wide borders.
    in2p = nc.dram_tensor("in2p", [C, hp, WP], FP16, kind="ExternalInput")
    out = nc.dram_tensor("out", [K, h, W], FP16, kind="ExternalOutput")

    with TileContext(nc) as tc:
        with (
            tc.tile_pool(name="cst", bufs=1) as cst,
            tc.tile_pool(name="s1p", bufs=1) as s1p,
            tc.tile_pool(name="wp", bufs=2) as wp,
            tc.tile_pool(name="t2xp", bufs=2) as t2xp,
            tc.tile_pool(name="t2p", bufs=2) as t2p,
            tc.tile_pool(name="top", bufs=2) as top,
            tc.tile_pool(name="gpp", bufs=3, space="PSUM") as gpp,
            tc.tile_pool(name="ttp", bufs=2, space="PSUM") as ttp,
        ):
            s2p = cst.tile([C, hp, WP], FP16, name="s2p")
            ident = cst.tile([128, 128], FP16, name="ident")

            # Interleave input loads so chunk-0 compute starts earliest:
            # s1c0, in2p rows for blocks 0-1, rows for blocks 2-3, s1c1, ...
            s1cs = []
            row_hi = 0

            def _load_s1c(ch):
                s1c = s1p.tile(
                    [C, NBLK, NG, 128], FP16, name=f"s1c{ch}", tag=f"s1c{ch}"
                )
                b0 = ch * NBLK
                if ch == 0:
                    # chunk 0 on Sync (overlaps the first in2p slice on the
                    # GpSimd ring), block 0 first so MMs start earliest
                    nc.sync.dma_start(s1c[:, 0:1, :, :], in1s[:, b0 : b0 + 1, :, :])
                    nc.sync.dma_start(
                        s1c[:, 1:, :, :], in1s[:, b0 + 1 : b0 + NBLK, :, :]
                    )
                else:
                    nc.gpsimd.dma_start(
                        s1c[:, :, :, :], in1s[:, b0 : b0 + NBLK, :, :]
                    )
                s1cs.append(s1c)

            def _load_s2p(need):
                nonlocal row_hi
                need = min(need, hp)
                if need > row_hi:
                    nc.gpsimd.dma_start(
                        s2p[:, row_hi:need, :], in2p[:, row_hi:need, :]
                    )
                    row_hi = need

            _load_s1c(0)
            _load_s2p(BR + 2 * PAD)  # rows for chunk-0 block 0
            _load_s2p(2 * BR + 2 * PAD)  # block 1
            _load_s2p(RCH + 2 * PAD)  # rest of chunk 0
            for ch in range(1, nch):
                _load_s1c(ch)
                _load_s2p(RCH * (ch + 1) + 2 * PAD if ch < nch - 1 else hp)

            make_identity(nc, ident)

            def head(ch):
                """MMs + psum drains + extraction DMAs for chunk ch."""
                y0 = ch * RCH
                s1c = s1cs[ch]

                # W[m | dy', v, blkg] fp16, blkg = 8*blk + g innermost
                wt = wp.tile([128, DYP, VP, NBG], FP16, name="wt", tag="wt")
                for blk in range(NBLK):
                    yb = y0 + BR * blk  # top row of block, in padded coords
                    for half in range(NG // 2):
                        gp = gpp.tile([128, 2, 512], FP32, name="gp", tag="gp")
                        for j in range(2):
                            g = 2 * half + j
                            nc.tensor.matmul(
                                gp[:, j, 0:NMM].rearrange(
                                    "p (dy v) -> p dy v", dy=DYP
                                ),
                                s1c[:, blk, g, :],
                                s2p[:, yb : yb + DYP, BU * g : BU * g + VP],
                                start=True,
                                stop=True,
                            )
                        # PSUM -> W (fp32 -> fp16); innermost j-pair contiguous
                        bg = NG * blk + 2 * half
                        dst = wt[:, :, :, bg : bg + 2]
                        src = gp[:, :, 0:NMM].rearrange(
                            "p j (dy v) -> p dy v j", dy=DYP
                        )
                        i = blk * 4 + half
                        if i % 2 == 1:
                            nc.vector.tensor_copy(dst, src)
                        else:
                            nc.scalar.activation(dst, src, COPY)

                # --- u-extract (x-shift): 16 DMAs on Sync, 576B runs ---
                # t2x[m | dy', dx, blkg] = wt[m | dy', u+dx, blkg], u = m%16
                t2x = t2xp.tile([128, DYP, D, NBG], FP16, name="t2x", tag="t2x")
                for u in range(BU):
                    nc.sync.dma_start(
                        t2x[u::BU, :, :, :],
                        wt[u::BU, :, u : u + D, :],
                    )

                # --- r-unshift (y-shift): 8 DMAs, 5.2KB runs ---
                # t2f[16r+u | dy, dx, blkg] = t2x[16r+u | r+dy, dx, blkg]
                # Last chunk: Sync (it is idle by then and skips the SWDGE
                # queue-drain stall on the exposed final tail).
                t2f = t2p.tile([128, D, D, NBG], FP16, name="t2f", tag="t2f")
                for r in range(BR):
                    if ch == nch - 1:
                        eng2 = nc.sync if r % 2 == 0 else nc.scalar
                    else:
                        eng2 = nc.gpsimd
                    eng2.dma_start(
                        t2f[BU * r : BU * r + BU, :, :, :],
                        t2x[BU * r : BU * r + BU, r : r + D, :, :],
                    )
                return t2f

            def tail(ch, t2f):
                """Reorder + PE transpose + scaled drain + store for chunk ch."""
                y0 = ch * RCH
                last = ch == nch - 1

                to = top.tile([K, RCH, W], FP16, name="to", tag="to")
                for blk in range(NBLK):
                    for hf in range(2):
                        tt = ttp.tile([K, 4, 128], FP16, name="tt", tag="tt")
                        for gi in range(4):
                            g = 4 * hf + gi
                            bg = NG * blk + g
                            # stationary: one free dim of 81 elems with
                            # uniform 64B stride (blkg is innermost in t2f)
                            nc.tensor.transpose(
                                tt[:, gi, :],
                                t2f[:, :, :, bg].rearrange("p dy dx -> p (dy dx)"),
                                ident[:, :],
                            )
                        # to[k, 8blk+r, 16g+u] <- tt[k, gi, 16r+u]
                        dst = to[
                            :, BR * blk : BR * blk + BR, 64 * hf : 64 * hf + 64
                        ].rearrange("k r (g u) -> k r g u", g=4)
                        src = tt[:, :, :].rearrange("k g (r u) -> k r g u", r=BR)
                        if (blk * 2 + hf) % 2 == 0:
                            nc.scalar.activation(dst, src, COPY, scale=1.0 / C)
                        else:
                            nc.vector.tensor_scalar_mul(dst, src, 1.0 / C)
                    if last and blk == NBLK // 2 - 1:
                        # pipeline the exposed final store with the drains
                        nc.sync.dma_start(
                            out[:, y0 : y0 + RCH // 2, :],
                            to[:, 0 : RCH // 2, :],
                        )

                # --- store: contiguous fp16 block ---
                if last:
                    nc.sync.dma_start(
                        out[:, y0 + RCH // 2 : y0 + RCH, :],
                        to[:, RCH // 2 :, :],
                    )
                else:
                    nc.sync.dma_start(out[:, y0 : y0 + RCH, :], to[:, :, :])

            # Software-pipelined emission: chunk ch's tail is emitted after
            # chunk ch+1's head so each engine's static schedule interleaves
            # the extraction chain of one chunk with the compute of the next.
            prev = None
            for ch in range(nch):
                t2f = head(ch)
                if prev is not None:
                    tail(ch - 1, prev)
                prev = t2f
            tail(nch - 1, prev)

    nc.compile()
    return nc


_cached = {}


def _get_nc(h: int):
    if h not in _cached:
        _cached[h] = build_bass(h)
    return _cached[h]


def _pad_in2(in2: np.ndarray) -> np.ndarray:
    # [C, h, W] fp16 -> [C, h+8, W+8] zero-padded, contiguous fp16
    return np.pad(
        in2.astype(np.float16), ((0, 0), (PAD, PAD), (PAD, PAD)), mode="constant"
    )


def _shuffle_in1(in1: np.ndarray) -> np.ndarray:
    # [C, h, W] -> [C, h//8, 8(g), 128(m)] with m = 16r+u,
    # y = 8*blk + r, x = 16*g + u.
    c, h, w = in1.shape
    a = in1.astype(np.float16).reshape(c, h // BR, BR, NG, BU)  # c,blk,r,g,u
    a = a.transpose(0, 1, 3, 2, 4)  # c, blk, g, r, u
    return np.ascontiguousarray(a.reshape(c, h // BR, NG, 128))


def kernel(**inputs: np.ndarray) -> np.ndarray:
    in1 = np.asarray(inputs["in1"], dtype=np.float32)
    in2 = np.asarray(inputs["in2"], dtype=np.float32)
    assert in1.shape == (B, C, H, W), in1.shape

    nc = _get_nc(H)
    in_maps = [
        {
            "in1s": _shuffle_in1(in1[b]),
            "in2p": np.ascontiguousarray(_pad_in2(in2[b])),
        }
        for b in range(B)
    ]
    res = run_bass_kernel_spmd(nc, in_maps, core_ids=list(range(N_CORES)))
    return np.stack([r["out"] for r in res.results], axis=0).astype(np.float32)


# revision 37
# speedup vs baseline: 1.3507x; 1.3507x over previous
"""Correlation cost-volume kernel for Trainium2 (Bass/Tile), v3.

Problem: in1, in2: [B=8, C=128, H=96, W=128] fp32.
Output: [B, 81, H, W] where out[b, dy*9+dx, y, x] =
    mean_c( in1[b,c,y,x] * in2_pad[b,c,y+dy,x+dx] ),
with in2 zero-padded by 4 in both spatial dims (max_displacement=4).

Data-parallel over batch (one sample per NeuronCore), fp16 compute.

Pipeline per 32-row chunk (4 row-blocks x 8 col-groups of 8x16 pixels,
m = 16r+u):
  1. matmul per block: stationary in1[:, 8rows x 16px] (M=128), moving
     in2p[:, 16 dy', 24 v] (N=384) -> psum[m, dy', v].
  2. ACT/DVE drain psum -> W[m | dy'16, v24, blkg32] fp16 (blkg = 8blk+g
     innermost so the extraction DMAs below get 576B contiguous runs).
  3. u-extract (x-shift): 16 DMAs on Sync (one per u, partitions u::16):
     t2x[m | dy', dx, blkg] = W[m | dy', u+dx, blkg]; src/dst runs 576B+.
  4. r-unshift (y-shift): 8 DMAs on GpSimd (one per r, partitions
     16r..16r+16): t2f[m | dy, dx, blkg] = t2x[m | r+dy, dx, blkg]; one
     contiguous 5.2KB run per partition.
  5. DVE reorder t2f -> t3[m | blkg, 81] so the PE transpose stationary
     has a single contiguous free dim.
  6. PE transpose per block-group: tt[k=81, m] <- t3[:, bg, :].
  7. ACT/DVE drain with 1/C scale to fp16 staging, one contiguous store
     per chunk; host upcasts to fp32.

v3 vs v2: u-first extraction (1.18+0.66 MB/chunk vs 1.77+0.66), Scalar
does no DMAs (was starving the psum drains), all stationary loads are
prefetched upfront, in2p load slices match per-chunk consumption so
chunk-0 matmuls start ~6us in, PSUM pools 3x2-bank mm tiles + 2x1-bank
transpose tiles.
"""

import numpy as np

import concourse.bass as bass
import concourse.mybir as mybir
from concourse import bacc
from concourse.bass_utils import run_bass_kernel_spmd
from concourse.masks import make_identity
from concourse.tile import TileContext

B = 8
C = 128
H = 96
W = 128
D = 9  # 2*max_disp + 1
K = D * D  # 81 output channels
PAD = 4
WP = W + 2 * PAD  # 136
FP32 = mybir.dt.float32
FP16 = mybir.dt.float16

N_CORES = 8
RCH = 32  # rows per chunk
BR = 8  # block rows
BU = 16  # block cols
NBLK = RCH // BR  # 4 row-blocks per chunk
NG = W // BU  # 8 col-groups
NBG = NBLK * NG  # 32 pixel-blocks per chunk
DYP = BR + 2 * PAD  # 16 dy' values per block
VP = BU + 2 * PAD  # 24 v values per group
NMM = DYP * VP  # 384 matmul free size
COPY = mybir.ActivationFunctionType.Copy


def build_bass(h: int = H):
    """Build the per-core Bass program for a [C, h, W] sample."""
    hp = h + 2 * PAD
    nch = h // RCH
    assert h % RCH == 0
    nc = bacc.Bacc(None, target_bir_lowering=False)
    # in1s is host-shuffled to [C, blk_total, g, m] with m = 16r+u,
    # y = 8*blk + r, x = 16*g + u (stationary needs one free dim).
    in1s = nc.dram_tensor("in1s", [C, h // BR, NG, 128], FP16, kind="ExternalInput")
    # in2p is host-padded: [C, h+8, W+8] with zeros in the 4-wide borders.
    in2p = nc.dram_tensor("in2p", [C, hp, WP], FP16, kind="ExternalInput")
    out = nc.dram_tensor("out", [K, h, W], FP16, kind="ExternalOutput")

    with TileContext(nc) as tc:
        with (
            tc.tile_pool(name="cst", bufs=1) as cst,
            tc.tile_pool(name="s1p", bufs=1) as s1p,
            tc.tile_pool(name="wp", bufs=2) as wp,
            tc.tile_pool(name="t2xp", bufs=2) as t2xp,
            tc.tile_pool(name="t2p", bufs=2) as t2p,
            tc.tile_pool(name="top", bufs=2) as top,
            tc.tile_pool(name="gpp", bufs=3, space="PSUM") as gpp,
            tc.tile_pool(name="ttp", bufs=2, space="PSUM") as ttp,
        ):
            s2p = cst.tile([C, hp, WP], FP16, name="s2p")
            ident = cst.tile([128, 128], FP16, name="ident")

            # Interleave input loads so chunk-0 compute starts earliest:
            # s1c0, in2p rows for blocks 0-1, rows for blocks 2-3, s1c1, ...
            s1cs = []
            row_hi = 0

            def _load_s1c(ch):
                s1c = s1p.tile(
                    [C, NBLK, NG, 128], FP16, name=f"s1c{ch}", tag=f"s1c{ch}"
                )
                b0 = ch * NBLK
                if ch == 0:
                    # chunk 0 on Sync (overlaps the first in2p slice on the
                    # GpSimd ring), block 0 first so MMs start earliest
                    nc.sync.dma_start(s1c[:, 0:1, :, :], in1s[:, b0 : b0 + 1, :, :])
                    nc.sync.dma_start(
                        s1c[:, 1:, :, :], in1s[:, b0 + 1 : b0 + NBLK, :, :]
                    )
                else:
                    nc.gpsimd.dma_start(
                        s1c[:, :, :, :], in1s[:, b0 : b0 + NBLK, :, :]
                    )
                s1cs.append(s1c)

            def _load_s2p(need):
                nonlocal row_hi
                need = min(need, hp)
                if need > row_hi:
                    nc.gpsimd.dma_start(
                        s2p[:, row_hi:need, :], in2p[:, row_hi:need, :]
                    )
                    row_hi = need

            _load_s1c(0)
            _load_s2p(BR + 2 * PAD)  # rows for chunk-0 block 0
            _load_s2p(2 * BR + 2 * PAD)  # block 1
            _load_s2p(RCH + 2 * PAD)  # rest of chunk 0
            for ch in range(1, nch):
                _load_s1c(ch)
                _load_s2p(RCH * (ch + 1) + 2 * PAD if ch < nch - 1 else hp)

            make_identity(nc, ident)

            def head(ch):
                """MMs + psum drains + extraction DMAs for chunk ch."""
                y0 = ch * RCH
                s1c = s1cs[ch]

                # W[m | dy', v, blkg] fp16, blkg = 8*blk + g innermost
                wt = wp.tile([128, DYP, VP, NBG], FP16, name="wt", tag="wt")
                for blk in range(NBLK):
                    yb = y0 + BR * blk  # top row of block, in padded coords
                    for half in range(NG // 2):
                        gp = gpp.tile([128, 2, 512], FP32, name="gp", tag="gp")
                        for j in range(2):
                            g = 2 * half + j
                            nc.tensor.matmul(
                                gp[:, j, 0:NMM].rearrange(
                                    "p (dy v) -> p dy v", dy=DYP
                                ),
                                s1c[:, blk, g, :],
                                s2p[:, yb : yb + DYP, BU * g : BU * g + VP],
                                start=True,
                                stop=True,
                            )
                        # PSUM -> W (fp32 -> fp16); innermost j-pair contiguous
                        bg = NG * blk + 2 * half
                        dst = wt[:, :, :, bg : bg + 2]
                        src = gp[:, :, 0:NMM].rearrange(
                            "p j (dy v) -> p dy v j", dy=DYP
                        )
                        i = blk * 4 + half
                        if i % 2 == 1:
                            nc.vector.tensor_copy(dst, src)
                        else:
                            nc.scalar.activation(dst, src, COPY)

                # --- u-extract (x-shift): 16 DMAs on Sync, 576B runs ---
                # t2x[m | dy', dx, blkg] = wt[m | dy', u+dx, blkg], u = m%16
                t2x = t2xp.tile([128, DYP, D, NBG], FP16, name="t2x", tag="t2x")
                for u in range(BU):
                    nc.sync.dma_start(
                        t2x[u::BU, :, :, :],
                        wt[u::BU, :, u : u + D, :],
                    )

                # --- r-unshift (y-shift): 8 DMAs, 5.2KB runs ---
                # t2f[16r+u | dy, dx, blkg] = t2x[16r+u | r+dy, dx, blkg]
                # Last chunk: Sync (it is idle by then and skips the SWDGE
                # queue-drain stall on the exposed final tail).
                t2f = t2p.tile([128, D, D, NBG], FP16, name="t2f", tag="t2f")
                for r in range(BR):
                    if ch == nch - 1:
                        eng2 = nc.sync if r % 2 == 0 else nc.scalar
                    else:
                        eng2 = nc.gpsimd
                    eng2.dma_start(
                        t2f[BU * r : BU * r + BU, :, :, :],
                        t2x[BU * r : BU * r + BU, r : r + D, :, :],
                    )
                return t2f

            def tail(ch, t2f):
                """Reorder + PE transpose + scaled drain + store for chunk ch."""
                y0 = ch * RCH
                last = ch == nch - 1

                to = top.tile([K, RCH, W], FP16, name="to", tag="to")
                for blk in range(NBLK):
                    for hf in range(2):
                        tt = ttp.tile([K, 4, 128], FP16, name="tt", tag="tt")
                        for gi in range(4):
                            g = 4 * hf + gi
                            bg = NG * blk + g
                            # stationary: one free dim of 81 elems with
                            # uniform 64B stride (blkg is innermost in t2f)
                            nc.tensor.transpose(
                                tt[:, gi, :],
                                t2f[:, :, :, bg].rearrange("p dy dx -> p (dy dx)"),
                                ident[:, :],
                            )
                        # to[k, 8blk+r, 16g+u] <- tt[k, gi, 16r+u]
                        dst = to[
                            :, BR * blk : BR * blk + BR, 64 * hf : 64 * hf + 64
                        ].rearrange("k r (g u) -> k r g u", g=4)
                        src = tt[:, :, :].rearrange("k g (r u) -> k r g u", r=BR)
                        if (blk * 2 + hf) % 2 == 0:
                            nc.scalar.activation(dst, src, COPY, scale=1.0 / C)
                        else:
                            nc.vector.tensor_scalar_mul(dst, src, 1.0 / C)
                    if last and blk == NBLK // 2 - 1:
                        # pipeline the exposed final store with the drains
                        nc.sync.dma_start(
                            out[:, y0 : y0 + RCH // 2, :],
                            to[:, 0 : RCH // 2, :],
                        )

                # --- store: contiguous fp16 block ---
                if last:
                    nc.sync.dma_start(
                        out[:, y0 + RCH // 2 : y0 + RCH, :],
                        to[:, RCH // 2 :, :],
                    )
                else:
                    nc.sync.dma_start(out[:, y0 : y0 + RCH, :], to[:, :, :])

            # Software-pipelined emission: chunk ch's tail is emitted after
            # chunk ch+1's head so each engine's static schedule interleaves
            # the extraction chain of one chunk with the compute of the next.
            prev = None
            for ch in range(nch):
                t2f = head(ch)
                if prev is not None:
                    tail(ch - 1, prev)
                prev = t2f
            tail(nch - 1, prev)

    nc.compile()
    return nc


_cached = {}


def _get_nc(h: int):
    if h not in _cached:
        _cached[h] = build_bass(h)
    return _cached[h]


def _pad_in2(in2: np.ndarray) -> np.ndarray:
    # [C, h, W] fp16 -> [C, h+8, W+8] zero-padded, contiguous fp16
    return np.pad(
        in2.astype(np.float16), ((0, 0), (PAD, PAD), (PAD, PAD)), mode="constant"
    )


def _shuffle_in1(in1: np.ndarray) -> np.ndarray:
    # [C, h, W] -> [C, h//8, 8(g), 128(m)] with m = 16r+u,
    # y = 8*blk + r, x = 16*g + u.
    c, h, w = in1.shape
    a = in1.astype(np.float16).reshape(c, h // BR, BR, NG, BU)  # c,blk,r,g,u
    a = a.transpose(0, 1, 3, 2, 4)  # c, blk, g, r, u
    return np.ascontiguousarray(a.reshape(c, h // BR, NG, 128))


def kernel(**inputs: np.ndarray) -> np.ndarray:
    in1 = np.asarray(inputs["in1"], dtype=np.float32)
    in2 = np.asarray(inputs["in2"], dtype=np.float32)
    assert in1.shape == (B, C, H, W), in1.shape

    nc = _get_nc(H)
    in_maps = [
        {
            "in1s": _shuffle_in1(in1[b]),
            "in2p": np.ascontiguousarray(_pad_in2(in2[b])),
        }
        for b in range(B)
    ]
    res = run_bass_kernel_spmd(nc, in_maps, core_ids=list(range(N_CORES)))
    return np.stack([r["out"] for r in res.results], axis=0).astype(np.float32)
